# revision 20
# baseline (speedup 1.0000x reference)
"""GQA attention (RoPE, no mask) sharded over 8 NeuronCores.

Sharding: TP over the 4 KV-head groups x DP over batch (2).
core c -> batch b = c//4, kv-group g = c%4 (query heads 4g..4g+3).
Each core computes Q/K/V projections for its heads, RoPE, softmax(QK^T)V,
and its o_proj partial; the 4 partials per batch are summed host-side.

Design notes (HW-validated best variant; 437-470us baseline -> ~388us):
- fp16 storage for X/W/q/k/v/ot (6x lower quantization error than bf16,
  same PE speed); probs stay bf16 for exp range (logits ~ +-50).
- Scores computed K-major per head-PAIR into one [128,1024] f32 PSUM
  (2 banks); ONE exp per pair halves ACT instruction overhead.
- Softmax denominators: DVE bf16 adds (2x 16-bit rate) into a [128,1024]
  accumulator, partition-reduced by two cheap ones-matmuls at block end,
  reciprocal in bf16, gpsimd partition_broadcast, DVE scale. (gpsimd
  partition_all_reduce is 13us/call on HW - do not use; PE K=1 broadcast
  matmuls and gpsimd adds in the loop also measured slower.)
- V^T computed directly (lhsT = X chunk, rhs = Wv) - no PE transposes.
- o_proj(qc-1) and qproj(qc+1) matmuls run as dense bursts at qc
  boundaries (A/B on HW showed in-loop drip-feeding them is slightly
  slower - HW favors uninterrupted engine streams).
- PSUM: "st" [128,1024] f32 x2 (8KB) + "ot" [128,512] f32 x4 (8KB) = 16KB.
- DMA queues: SP=xkv/xq0/out, Pool=weights+tables+xq prefetch.
"""

import sys

sys.path.insert(0, "/opt/trn_rl_repo")

from contextlib import ExitStack

import numpy as np

import concourse.bass as bass
import concourse.tile as tile
from concourse import bacc, bass_isa, mybir
from concourse.bass_utils import run_bass_kernel_spmd

BF16 = mybir.dt.bfloat16
F16 = mybir.dt.float16
F32 = mybir.dt.float32
NP_F16 = np.float16

B, T_FULL, S_FULL, D_FULL = 2, 2048, 2048, 2048
N_HEADS, KV_HEADS, H = 16, 4, 128
HG = N_HEADS // KV_HEADS  # query heads per core (4)
HD = HG * H  # per-core q head dims (512)
MIN_TS, MAX_TS = 1.0, 10000.0


def build(T=T_FULL, S=S_FULL, D=D_FULL, repeat=1):
    """Build the per-core Bass graph. Returns compiled nc."""
    assert T % 512 == 0 and S % 512 == 0 and D % 128 == 0
    TQC = T // 512  # q chunks of 512
    SC = S // 512  # kv chunks of 512
    S128 = S // 128  # kv chunks of 128
    DC = D // 128  # contraction chunks of 128

    nc = bacc.Bacc("TRN2", target_bir_lowering=False, debug=False, num_devices=8)

    # Host-prelayouted inputs; every DMA is contiguous per partition.
    xq_d = nc.dram_tensor("XqT", [TQC, 128, DC, 512], F16, kind="ExternalInput").ap()
    xkv_d = nc.dram_tensor("XkvT", [SC, 128, DC, 512], F16, kind="ExternalInput").ap()
    wq_d = nc.dram_tensor("Wq", [128, DC, HD], F16, kind="ExternalInput").ap()
    wk_d = nc.dram_tensor("Wk", [128, DC, H], F16, kind="ExternalInput").ap()
    wv_d = nc.dram_tensor("Wv", [128, DC, H], F16, kind="ExternalInput").ap()
    wo_d = nc.dram_tensor("Wo", [128, HG, D], F16, kind="ExternalInput").ap()
    cosq_d = nc.dram_tensor("cos_q", [H // 2, T], F32, kind="ExternalInput").ap()
    sinq_d = nc.dram_tensor("sin_q", [H // 2, T], F32, kind="ExternalInput").ap()
    cosk_d = nc.dram_tensor("cos_k", [H // 2, S], F32, kind="ExternalInput").ap()
    sink_d = nc.dram_tensor("sin_k", [H // 2, S], F32, kind="ExternalInput").ap()
    out_d = nc.dram_tensor("out", [T, D], F16, kind="ExternalOutput").ap()

    with tile.TileContext(nc) as tc, ExitStack() as ctx:
        wpool = ctx.enter_context(tc.tile_pool(name="w", bufs=1))
        xpool = ctx.enter_context(tc.tile_pool(name="x", bufs=3))
        qkv = ctx.enter_context(tc.tile_pool(name="qkv", bufs=1))
        ptp = ctx.enter_context(tc.tile_pool(name="pt", bufs=4))
        accp = ctx.enter_context(tc.tile_pool(name="acc", bufs=2))
        tmpp = ctx.enter_context(tc.tile_pool(name="tmp", bufs=4))
        outp = ctx.enter_context(tc.tile_pool(name="outs", bufs=2))
        ps_st = ctx.enter_context(tc.tile_pool(name="ps_st", bufs=2, space="PSUM"))
        ps_ot = ctx.enter_context(tc.tile_pool(name="ps_ot", bufs=4, space="PSUM"))

        # ---- weights / tables on the Pool queue (KV path first; ACT and
        # SP queues are needed for exps / x chunks) ----
        wk_sb = wpool.tile([128, DC, H], F16, tag="wk")
        nc.gpsimd.dma_start(wk_sb[:], wk_d[:])
        wv_sb = wpool.tile([128, DC, H], F16, tag="wv")
        nc.gpsimd.dma_start(wv_sb[:], wv_d[:])
        # cos/sin packed: rows 0:64 = q tables, 64:128 = k tables
        cos_sb = wpool.tile([128, max(T, S)], F32, tag="cos")
        sin_sb = wpool.tile([128, max(T, S)], F32, tag="sin")
        nc.gpsimd.dma_start(cos_sb[64:128, 0:S], cosk_d[:])
        nc.gpsimd.dma_start(sin_sb[64:128, 0:S], sink_d[:])
        wq_sb = wpool.tile([128, DC, HD], F16, tag="wq")
        nc.gpsimd.dma_start(wq_sb[:], wq_d[:])
        nc.gpsimd.dma_start(cos_sb[0:64, 0:T], cosq_d[:])
        nc.gpsimd.dma_start(sin_sb[0:64, 0:T], sinq_d[:])
        wo_sb = wpool.tile([128, HG, D], F16, tag="wo")
        nc.gpsimd.dma_start(wo_sb[:], wo_d[:])

        ones_col = wpool.tile([128, 1], BF16, tag="ones_col")
        nc.vector.memset(ones_col[:], 1.0)
        qt_sb = qkv.tile([128, HG, T], F16, tag="qt")
        kt_sb = qkv.tile([128, S], F16, tag="kt")
        v_sb = qkv.tile([128, S], F16, tag="v")  # [s-in-block, S128*H] = V^T blocks
        ot_sb = qkv.tile([128, HG, T], F16, tag="ot")

        def rope(dst, ps, cos_ap, sin_ap):
            # dst[0:64] = ps[0:64]*cos - ps[64:128]*sin
            # dst[64:128] = ps[64:128]*cos + ps[0:64]*sin
            t1 = tmpp.tile([64, 512], F32, tag="t1")
            t2 = tmpp.tile([64, 512], F32, tag="t2")
            nc.vector.tensor_mul(t1[:], ps[0:64, 0:512], cos_ap)
            nc.vector.tensor_mul(t2[:], ps[64:128, 0:512], sin_ap)
            nc.vector.tensor_sub(dst[0:64, :], t1[:], t2[:])
            t3 = tmpp.tile([64, 512], F32, tag="t1")
            t4 = tmpp.tile([64, 512], F32, tag="t2")
            nc.vector.tensor_mul(t3[:], ps[64:128, 0:512], cos_ap)
            nc.vector.tensor_mul(t4[:], ps[0:64, 0:512], sin_ap)
            nc.vector.tensor_add(dst[64:128, :], t3[:], t4[:])

        def body():
            pending_qp = []  # qproj filler thunks (1 PE matmul each)
            pending_op = []  # o_proj filler thunks

            def drain(n_op=1, n_qp=1):
                for _ in range(n_op):
                    if pending_op:
                        pending_op.pop(0)()
                for _ in range(n_qp):
                    if pending_qp:
                        pending_qp.pop(0)()

            def drain_all():
                while pending_op or pending_qp:
                    drain()

            # ---- K/V projections ----
            for j in range(SC):
                xk = xpool.tile([128, DC, 512], F16, tag="x")
                if j == 0:
                    for dd in range(4):
                        nc.sync.dma_start(
                            xk[:, 4 * dd : 4 * (dd + 1), :],
                            xkv_d[0, :, 4 * dd : 4 * (dd + 1), :],
                        )
                else:
                    nc.sync.dma_start(xk[:], xkv_d[j])
                psk = ps_st.tile([128, 1024], F32, tag="st")
                for d in range(DC):
                    nc.tensor.matmul(
                        psk[:, 0:512], wk_sb[:, d, :], xk[:, d, :],
                        start=(d == 0), stop=(d == DC - 1),
                    )
                rope(
                    kt_sb[:, bass.ts(j, 512)], psk,
                    cos_sb[64:128, bass.ts(j, 512)], sin_sb[64:128, bass.ts(j, 512)],
                )
                # V^T direct: out[s-block, h] = sum_d Xkv[d, s]^T Wv[d, h]
                psv = ps_st.tile([128, 1024], F32, tag="st")
                for sb in range(4):
                    for d in range(DC):
                        nc.tensor.matmul(
                            psv[:, 128 * sb : 128 * (sb + 1)],
                            xk[:, d, 128 * sb : 128 * (sb + 1)],
                            wv_sb[:, d, :],
                            start=(d == 0), stop=(d == DC - 1),
                        )
                nc.vector.tensor_copy(v_sb[:, bass.ts(j, 512)], psv[:, 0:512])

            xq_tiles = {}

            def load_xq(qc, eng):
                t = xpool.tile([128, DC, 512], F16, tag="x")
                eng.dma_start(t[:], xq_d[qc])
                xq_tiles[qc] = t

            load_xq(0, nc.sync)

            def qproj_thunks(qc, hh):
                st = {}

                def mk(d):
                    def run():
                        if d == 0:
                            st["ps"] = ps_ot.tile([128, 512], F32, tag="ot", name=f"psq_{qc}_{hh}")
                        nc.tensor.matmul(
                            st["ps"][:], wq_sb[:, d, bass.ts(hh, 128)],
                            xq_tiles[qc][:, d, :],
                            start=(d == 0), stop=(d == DC - 1),
                        )
                        if d == DC - 1:
                            rope(
                                qt_sb[:, hh, bass.ts(qc, 512)], st["ps"],
                                cos_sb[0:64, bass.ts(qc, 512)],
                                sin_sb[0:64, bass.ts(qc, 512)],
                            )

                    return run

                return [mk(d) for d in range(DC)]

            # qproj(0): dense
            for hh in range(HG):
                for th in qproj_thunks(0, hh):
                    th()

            def oproj_thunks(qc):
                st = {}
                thunks = []
                for tsub in range(4):
                    for dc2 in range(D // 512):
                        for hh in range(HG):
                            def mk(tsub=tsub, dc2=dc2, hh=hh):
                                def run():
                                    trow = qc * 512 + tsub * 128
                                    if hh == 0:
                                        if dc2 == 0:
                                            st[tsub] = outp.tile(
                                                [128, D], F16, tag="ostage",
                                                name=f"ostage_{qc}_{tsub}",
                                            )
                                        st["ps"] = ps_ot.tile(
                                            [128, 512], F32, tag="ot",
                                            name=f"pso2_{qc}_{tsub}_{dc2}",
                                        )
                                    nc.tensor.matmul(
                                        st["ps"][:],
                                        ot_sb[:, hh, trow : trow + 128],
                                        wo_sb[:, hh, bass.ts(dc2, 512)],
                                        start=(hh == 0), stop=(hh == HG - 1),
                                    )
                                    if hh == HG - 1:
                                        if dc2 % 2 == 0:
                                            nc.scalar.copy(
                                                st[tsub][:, bass.ts(dc2, 512)],
                                                st["ps"][:],
                                            )
                                        else:
                                            nc.vector.tensor_copy(
                                                st[tsub][:, bass.ts(dc2, 512)],
                                                st["ps"][:],
                                            )
                                        if dc2 == D // 512 - 1:
                                            nc.sync.dma_start(
                                                out_d[trow : trow + 128, :],
                                                st[tsub][:],
                                            )

                                return run

                            thunks.append(mk())
                return thunks

            blk_state = {}

            def attn_prologue(qc, hp):
                h0, h1 = 2 * hp, 2 * hp + 1
                pso0 = ps_ot.tile([128, 512], F32, tag="ot", name=f"pso0_{qc}_{hp}")
                pso1 = ps_ot.tile([128, 512], F32, tag="ot", name=f"pso1_{qc}_{hp}")
                acc = accp.tile([128, 1024], BF16, tag="acc", name=f"acc_{qc}_{hp}")
                st_tiles = [None] * S128

                def emit_st(s):
                    pst = ps_st.tile([128, 1024], F32, tag="st")
                    nc.tensor.matmul(
                        pst[:, 0:512], kt_sb[:, bass.ts(s, 128)],
                        qt_sb[:, h0, bass.ts(qc, 512)], start=True, stop=True,
                    )
                    nc.tensor.matmul(
                        pst[:, 512:1024], kt_sb[:, bass.ts(s, 128)],
                        qt_sb[:, h1, bass.ts(qc, 512)], start=True, stop=True,
                    )
                    st_tiles[s] = pst

                emit_st(0)
                blk_state[(qc, hp)] = (pso0, pso1, acc, st_tiles, emit_st)

            def attn_block(qc, hp, qp_gate, op_gate=0):
                h0, h1 = 2 * hp, 2 * hp + 1
                if (qc, hp) not in blk_state:
                    attn_prologue(qc, hp)
                pso0, pso1, acc, st_tiles, emit_st = blk_state.pop((qc, hp))
                for s in range(S128):
                    if s + 1 < S128:
                        emit_st(s + 1)
                    pst = st_tiles[s]
                    st_tiles[s] = None
                    pt = ptp.tile([128, 1024], BF16, tag="pt")
                    nc.scalar.activation(
                        pt[:], pst[:], mybir.ActivationFunctionType.Exp
                    )
                    if s == 0:
                        nc.vector.tensor_copy(acc[:], pt[:])
                    else:
                        nc.vector.tensor_add(acc[:], acc[:], pt[:])
                    nc.tensor.matmul(
                        pso0[:], v_sb[:, bass.ts(s, 128)], pt[:, 0:512],
                        start=(s == 0), stop=(s == S128 - 1),
                    )
                    nc.tensor.matmul(
                        pso1[:], v_sb[:, bass.ts(s, 128)], pt[:, 512:1024],
                        start=(s == 0), stop=(s == S128 - 1),
                    )
                    pass  # v8 experiment: no in-loop filler draining
                # tail: evacuate PSUM fast, then partition-reduce the
                # denominators with two cheap ones-matmuls (dependency-ready
                # in the PE FIFO - no stall), reciprocal on DVE, broadcast on
                # gpsimd (bf16 to halve Q7 daisy-chain bytes), scale on DVE.
                po0 = accp.tile([128, 512], F32, tag="po0")
                po1 = accp.tile([128, 512], F32, tag="po1")
                nc.vector.tensor_copy(po0[:], pso0[:])
                nc.vector.tensor_copy(po1[:], pso1[:])
                psum = ps_st.tile([128, 1024], F32, tag="st", name=f"psum_{qc}_{hp}")
                nc.tensor.matmul(
                    psum[0:1, 0:512], ones_col[:], acc[:, 0:512],
                    start=True, stop=True,
                )
                nc.tensor.matmul(
                    psum[0:1, 512:1024], ones_col[:], acc[:, 512:1024],
                    start=True, stop=True,
                )
                rec = accp.tile([1, 1024], BF16, tag="rec")
                with nc.allow_low_precision(reason="softmax scale in bf16"):
                    nc.vector.reciprocal(rec[:], psum[0:1, 0:1024])
                rbc = accp.tile([128, 1024], BF16, tag="rbc")
                nc.gpsimd.partition_broadcast(rbc[:], rec[:])
                nc.vector.tensor_mul(
                    ot_sb[:, h0, bass.ts(qc, 512)], po0[:], rbc[:, 0:512]
                )
                nc.vector.tensor_mul(
                    ot_sb[:, h1, bass.ts(qc, 512)], po1[:], rbc[:, 512:1024]
                )

            for qc in range(TQC):
                if qc + 1 < TQC:
                    load_xq(qc + 1, nc.gpsimd)
                attn_block(qc, 0, qp_gate=S128)
                attn_block(qc, 1, qp_gate=0)
                if qc + 1 < TQC:
                    # boundary bursts: qproj heads 0,1 first, then hoist the
                    # next block's first score pair so ACT keeps working,
                    # then the remaining proj/o_proj work.
                    for hh in (0, 1):
                        for th in qproj_thunks(qc + 1, hh):
                            th()
                    attn_prologue(qc + 1, 0)
                    for hh in (2, 3):
                        for th in qproj_thunks(qc + 1, hh):
                            th()
                drain_all()
                pending_op.extend(oproj_thunks(qc))
            drain_all()

        if repeat == 1:
            body()
        else:
            with tc.For_i(0, repeat):
                body()

    nc.compile()
    return nc


def _shard_inputs(Xq, Xkv, q_positions, kv_positions, Wq, Wk, Wv, Wo):
    """Build per-core input maps. Core c: batch c//4, kv-group c%4."""
    D = Xq.shape[2]
    half = H // 2
    frac = 2.0 * np.arange(half, dtype=np.float32) / H
    ts = (MIN_TS * (MAX_TS / MIN_TS) ** frac).astype(np.float32)

    def tables(pos):
        s = pos.astype(np.float32)[None, :] / ts[:, None]
        return np.cos(s).astype(np.float32), np.sin(s).astype(np.float32)

    DC = D // 128

    def chunked_xT(X):
        # [L, D] -> X.T laid out as [L//512, 128, DC, 512]: contiguous per partition
        xt = np.ascontiguousarray(X.T).astype(NP_F16)  # [D, L]
        L = X.shape[0]
        return np.ascontiguousarray(
            xt.reshape(DC, 128, L // 512, 512).transpose(2, 1, 0, 3)
        )

    def chunked_w(W, m):
        # [D, m] -> [128, DC, m]
        return np.ascontiguousarray(
            W.reshape(DC, 128, m).transpose(1, 0, 2)
        ).astype(NP_F16)

    in_maps = []
    for c in range(8):
        b, g = c // 4, c % 4
        cq, sq = tables(q_positions[b])
        ck, sk = tables(kv_positions[b])
        in_maps.append(
            {
                "XqT": chunked_xT(Xq[b]),
                "XkvT": chunked_xT(Xkv[b]),
                "Wq": chunked_w(Wq[:, HG * g : HG * (g + 1), :].reshape(D, HD), HD),
                "Wk": chunked_w(Wk[:, g, :], H),
                "Wv": chunked_w(Wv[:, g, :], H),
                # Wo [HG, 128, D] -> [128, HG, D]: wo_sb[h, hh, d] = Wo[g*HG+hh, h, d]
                "Wo": np.ascontiguousarray(
                    Wo[HG * g : HG * (g + 1)].transpose(1, 0, 2)
                ).astype(NP_F16),
                "cos_q": cq, "sin_q": sq, "cos_k": ck, "sin_k": sk,
            }
        )
    return in_maps


_NC_CACHE = {}


def kernel(Xq, Xkv, q_positions, kv_positions, Wq, Wk, Wv, Wo):
    key = ("full", 1)
    if key not in _NC_CACHE:
        _NC_CACHE[key] = build()
    nc = _NC_CACHE[key]
    in_maps = _shard_inputs(Xq, Xkv, q_positions, kv_positions, Wq, Wk, Wv, Wo)
    res = run_bass_kernel_spmd(nc, in_maps, core_ids=list(range(8)))
    T, D = Xq.shape[1], Xq.shape[2]
    out = np.zeros((B, T, D), dtype=np.float32)
    for c in range(8):
        out[c // 4] += res.results[c]["out"].astype(np.float32)
    return out


# revision 21
# speedup vs baseline: 1.1083x; 1.1083x over previous
"""GQA attention (RoPE, no mask) sharded over 8 NeuronCores.

Sharding: TP over the 4 KV-head groups x DP over batch (2).
core c -> batch b = c//4, kv-group g = c%4 (query heads 4g..4g+3).
Each core computes Q/K/V projections for its heads, RoPE, softmax(QK^T)V,
and its o_proj partial; the 4 partials per batch are summed host-side.

Design notes (HW-validated best variant; 437-470us baseline -> ~388us):
- fp16 storage for X/W/q/k/v/ot (6x lower quantization error than bf16,
  same PE speed); probs stay bf16 for exp range (logits ~ +-50).
- Scores computed K-major per head-PAIR into one [128,1024] f32 PSUM
  (2 banks); ONE exp per pair halves ACT instruction overhead.
- Softmax denominators: DVE bf16 adds (2x 16-bit rate) into a [128,1024]
  accumulator, partition-reduced by two cheap ones-matmuls at block end,
  reciprocal in bf16, gpsimd partition_broadcast, DVE scale. (gpsimd
  partition_all_reduce is 13us/call on HW - do not use; PE K=1 broadcast
  matmuls and gpsimd adds in the loop also measured slower.)
- V^T computed directly (lhsT = X chunk, rhs = Wv) - no PE transposes.
- o_proj(qc-1) and qproj(qc+1) matmuls run as dense bursts at qc
  boundaries (A/B on HW showed in-loop drip-feeding them is slightly
  slower - HW favors uninterrupted engine streams).
- PSUM: "st" [128,1024] f32 x2 (8KB) + "ot" [128,512] f32 x4 (8KB) = 16KB.
- DMA queues: SP=xkv/xq0/out, Pool=weights+tables+xq prefetch.
"""

import sys

sys.path.insert(0, "/opt/trn_rl_repo")

from contextlib import ExitStack

import numpy as np

import concourse.bass as bass
import concourse.tile as tile
from concourse import bacc, bass_isa, mybir
from concourse.bass_utils import run_bass_kernel_spmd

BF16 = mybir.dt.bfloat16
F16 = mybir.dt.float16
F32 = mybir.dt.float32
NP_F16 = np.float16

B, T_FULL, S_FULL, D_FULL = 2, 2048, 2048, 2048
N_HEADS, KV_HEADS, H = 16, 4, 128
HG = N_HEADS // KV_HEADS  # query heads per core (4)
HD = HG * H  # per-core q head dims (512)
MIN_TS, MAX_TS = 1.0, 10000.0


def build(T=T_FULL, S=S_FULL, D=D_FULL, repeat=1):
    """Build the per-core Bass graph. Returns compiled nc."""
    assert T % 512 == 0 and S % 512 == 0 and D % 128 == 0
    TQC = T // 512  # q chunks of 512
    SC = S // 512  # kv chunks of 512
    S128 = S // 128  # kv chunks of 128
    DC = D // 128  # contraction chunks of 128

    nc = bacc.Bacc("TRN2", target_bir_lowering=False, debug=False, num_devices=8)

    # Host-prelayouted inputs; every DMA is contiguous per partition.
    xq_d = nc.dram_tensor("XqT", [TQC, 128, DC, 512], F16, kind="ExternalInput").ap()
    xkv_d = nc.dram_tensor("XkvT", [SC, 128, DC, 512], F16, kind="ExternalInput").ap()
    wq_d = nc.dram_tensor("Wq", [128, DC, HD], F16, kind="ExternalInput").ap()
    wk_d = nc.dram_tensor("Wk", [128, DC, H], F16, kind="ExternalInput").ap()
    wv_d = nc.dram_tensor("Wv", [128, DC, H], F16, kind="ExternalInput").ap()
    wo_d = nc.dram_tensor("Wo", [128, HG, D], F16, kind="ExternalInput").ap()
    cosq_d = nc.dram_tensor("cos_q", [H // 2, T], F32, kind="ExternalInput").ap()
    sinq_d = nc.dram_tensor("sin_q", [H // 2, T], F32, kind="ExternalInput").ap()
    cosk_d = nc.dram_tensor("cos_k", [H // 2, S], F32, kind="ExternalInput").ap()
    sink_d = nc.dram_tensor("sin_k", [H // 2, S], F32, kind="ExternalInput").ap()
    out_d = nc.dram_tensor("out", [T, D], F16, kind="ExternalOutput").ap()

    with tile.TileContext(nc) as tc, ExitStack() as ctx:
        wpool = ctx.enter_context(tc.tile_pool(name="w", bufs=1))
        xpool = ctx.enter_context(tc.tile_pool(name="x", bufs=3))
        qkv = ctx.enter_context(tc.tile_pool(name="qkv", bufs=1))
        ptp = ctx.enter_context(tc.tile_pool(name="pt", bufs=4))
        accp = ctx.enter_context(tc.tile_pool(name="acc", bufs=2))
        tmpp = ctx.enter_context(tc.tile_pool(name="tmp", bufs=4))
        outp = ctx.enter_context(tc.tile_pool(name="outs", bufs=2))
        ps_st = ctx.enter_context(tc.tile_pool(name="ps_st", bufs=2, space="PSUM"))
        ps_ot = ctx.enter_context(tc.tile_pool(name="ps_ot", bufs=4, space="PSUM"))

        # ---- weights / tables on the Pool queue (KV path first; ACT and
        # SP queues are needed for exps / x chunks) ----
        wk_sb = wpool.tile([128, DC, H], F16, tag="wk")
        nc.gpsimd.dma_start(wk_sb[:], wk_d[:])
        wv_sb = wpool.tile([128, DC, H], F16, tag="wv")
        nc.gpsimd.dma_start(wv_sb[:], wv_d[:])
        # cos/sin packed: rows 0:64 = q tables, 64:128 = k tables
        cos_sb = wpool.tile([128, max(T, S)], F32, tag="cos")
        sin_sb = wpool.tile([128, max(T, S)], F32, tag="sin")
        nc.gpsimd.dma_start(cos_sb[64:128, 0:S], cosk_d[:])
        nc.gpsimd.dma_start(sin_sb[64:128, 0:S], sink_d[:])
        wq_sb = wpool.tile([128, DC, HD], F16, tag="wq")
        nc.gpsimd.dma_start(wq_sb[:], wq_d[:])
        nc.gpsimd.dma_start(cos_sb[0:64, 0:T], cosq_d[:])
        nc.gpsimd.dma_start(sin_sb[0:64, 0:T], sinq_d[:])
        wo_sb = wpool.tile([128, HG, D], F16, tag="wo")
        nc.gpsimd.dma_start(wo_sb[:], wo_d[:])

        ones_col = wpool.tile([128, 1], BF16, tag="ones_col")
        nc.vector.memset(ones_col[:], 1.0)
        qt_sb = qkv.tile([128, HG, T], F16, tag="qt")
        kt_sb = qkv.tile([128, S], F16, tag="kt")
        v_sb = qkv.tile([128, S], F16, tag="v")  # [s-in-block, S128*H] = V^T blocks
        ot_sb = qkv.tile([128, HG, T], F16, tag="ot")

        def rope(dst, ps, cos_ap, sin_ap):
            # dst[0:64] = ps[0:64]*cos - ps[64:128]*sin
            # dst[64:128] = ps[64:128]*cos + ps[0:64]*sin
            t1 = tmpp.tile([64, 512], F32, tag="t1")
            t2 = tmpp.tile([64, 512], F32, tag="t2")
            nc.vector.tensor_mul(t1[:], ps[0:64, 0:512], cos_ap)
            nc.vector.tensor_mul(t2[:], ps[64:128, 0:512], sin_ap)
            nc.vector.tensor_sub(dst[0:64, :], t1[:], t2[:])
            t3 = tmpp.tile([64, 512], F32, tag="t1")
            t4 = tmpp.tile([64, 512], F32, tag="t2")
            nc.vector.tensor_mul(t3[:], ps[64:128, 0:512], cos_ap)
            nc.vector.tensor_mul(t4[:], ps[0:64, 0:512], sin_ap)
            nc.vector.tensor_add(dst[64:128, :], t3[:], t4[:])

        def body():
            pending_qp = []  # qproj filler thunks (1 PE matmul each)
            pending_op = []  # o_proj filler thunks

            def drain(n_op=1, n_qp=1):
                for _ in range(n_op):
                    if pending_op:
                        pending_op.pop(0)()
                for _ in range(n_qp):
                    if pending_qp:
                        pending_qp.pop(0)()

            def drain_all():
                while pending_op or pending_qp:
                    drain()

            # ---- K/V projections ----
            for j in range(SC):
                xk = xpool.tile([128, DC, 512], F16, tag="x")
                if j == 0:
                    for dd in range(4):
                        nc.sync.dma_start(
                            xk[:, 4 * dd : 4 * (dd + 1), :],
                            xkv_d[0, :, 4 * dd : 4 * (dd + 1), :],
                        )
                else:
                    nc.sync.dma_start(xk[:], xkv_d[j])
                psk = ps_st.tile([128, 1024], F32, tag="st")
                for d in range(DC):
                    nc.tensor.matmul(
                        psk[:, 0:512], wk_sb[:, d, :], xk[:, d, :],
                        start=(d == 0), stop=(d == DC - 1),
                    )
                rope(
                    kt_sb[:, bass.ts(j, 512)], psk,
                    cos_sb[64:128, bass.ts(j, 512)], sin_sb[64:128, bass.ts(j, 512)],
                )
                # V^T direct: out[s-block, h] = sum_d Xkv[d, s]^T Wv[d, h]
                psv = ps_st.tile([128, 1024], F32, tag="st")
                for sb in range(4):
                    for d in range(DC):
                        nc.tensor.matmul(
                            psv[:, 128 * sb : 128 * (sb + 1)],
                            xk[:, d, 128 * sb : 128 * (sb + 1)],
                            wv_sb[:, d, :],
                            start=(d == 0), stop=(d == DC - 1),
                        )
                nc.vector.tensor_copy(v_sb[:, bass.ts(j, 512)], psv[:, 0:512])

            xq_tiles = {}

            def load_xq(qc, eng):
                t = xpool.tile([128, DC, 512], F16, tag="x")
                eng.dma_start(t[:], xq_d[qc])
                xq_tiles[qc] = t

            load_xq(0, nc.sync)

            def qproj_thunks(qc, hh):
                st = {}

                def mk(d):
                    def run():
                        if d == 0:
                            st["ps"] = ps_ot.tile([128, 512], F32, tag="ot", name=f"psq_{qc}_{hh}")
                        nc.tensor.matmul(
                            st["ps"][:], wq_sb[:, d, bass.ts(hh, 128)],
                            xq_tiles[qc][:, d, :],
                            start=(d == 0), stop=(d == DC - 1),
                        )
                        if d == DC - 1:
                            rope(
                                qt_sb[:, hh, bass.ts(qc, 512)], st["ps"],
                                cos_sb[0:64, bass.ts(qc, 512)],
                                sin_sb[0:64, bass.ts(qc, 512)],
                            )

                    return run

                return [mk(d) for d in range(DC)]

            # qproj(0): dense
            for hh in range(HG):
                for th in qproj_thunks(0, hh):
                    th()

            def oproj_thunks(qc):
                st = {}
                thunks = []
                for tsub in range(4):
                    for dc2 in range(D // 512):
                        for hh in range(HG):
                            def mk(tsub=tsub, dc2=dc2, hh=hh):
                                def run():
                                    trow = qc * 512 + tsub * 128
                                    if hh == 0:
                                        if dc2 == 0:
                                            st[tsub] = outp.tile(
                                                [128, D], F16, tag="ostage",
                                                name=f"ostage_{qc}_{tsub}",
                                            )
                                        st["ps"] = ps_ot.tile(
                                            [128, 512], F32, tag="ot",
                                            name=f"pso2_{qc}_{tsub}_{dc2}",
                                        )
                                    nc.tensor.matmul(
                                        st["ps"][:],
                                        ot_sb[:, hh, trow : trow + 128],
                                        wo_sb[:, hh, bass.ts(dc2, 512)],
                                        start=(hh == 0), stop=(hh == HG - 1),
                                    )
                                    if hh == HG - 1:
                                        if dc2 % 2 == 0:
                                            nc.scalar.copy(
                                                st[tsub][:, bass.ts(dc2, 512)],
                                                st["ps"][:],
                                            )
                                        else:
                                            nc.vector.tensor_copy(
                                                st[tsub][:, bass.ts(dc2, 512)],
                                                st["ps"][:],
                                            )
                                        if dc2 == D // 512 - 1:
                                            nc.sync.dma_start(
                                                out_d[trow : trow + 128, :],
                                                st[tsub][:],
                                            )

                                return run

                            thunks.append(mk())
                return thunks

            def attn_block(qc, hp, qp_gate, op_gate=0):
                h0, h1 = 2 * hp, 2 * hp + 1
                pso0 = ps_ot.tile([128, 512], F32, tag="ot")
                pso1 = ps_ot.tile([128, 512], F32, tag="ot")
                acc = accp.tile([128, 1024], BF16, tag="acc")
                st_tiles = [None] * S128

                def emit_st(s):
                    pst = ps_st.tile([128, 1024], F32, tag="st")
                    nc.tensor.matmul(
                        pst[:, 0:512], kt_sb[:, bass.ts(s, 128)],
                        qt_sb[:, h0, bass.ts(qc, 512)], start=True, stop=True,
                    )
                    nc.tensor.matmul(
                        pst[:, 512:1024], kt_sb[:, bass.ts(s, 128)],
                        qt_sb[:, h1, bass.ts(qc, 512)], start=True, stop=True,
                    )
                    st_tiles[s] = pst

                emit_st(0)
                for s in range(S128):
                    if s + 1 < S128:
                        emit_st(s + 1)
                    pst = st_tiles[s]
                    st_tiles[s] = None
                    pt = ptp.tile([128, 1024], BF16, tag="pt")
                    nc.scalar.activation(
                        pt[:], pst[:], mybir.ActivationFunctionType.Exp
                    )
                    if s == 0:
                        nc.vector.tensor_copy(acc[:], pt[:])
                    else:
                        nc.vector.tensor_add(acc[:], acc[:], pt[:])
                    nc.tensor.matmul(
                        pso0[:], v_sb[:, bass.ts(s, 128)], pt[:, 0:512],
                        start=(s == 0), stop=(s == S128 - 1),
                    )
                    nc.tensor.matmul(
                        pso1[:], v_sb[:, bass.ts(s, 128)], pt[:, 512:1024],
                        start=(s == 0), stop=(s == S128 - 1),
                    )
                    pass  # v8 experiment: no in-loop filler draining
                # tail: evacuate PSUM fast, then partition-reduce the
                # denominators with two cheap ones-matmuls (dependency-ready
                # in the PE FIFO - no stall), reciprocal on DVE, broadcast on
                # gpsimd (bf16 to halve Q7 daisy-chain bytes), scale on DVE.
                po0 = accp.tile([128, 512], F32, tag="po0")
                po1 = accp.tile([128, 512], F32, tag="po1")
                nc.vector.tensor_copy(po0[:], pso0[:])
                nc.vector.tensor_copy(po1[:], pso1[:])
                psum = ps_st.tile([128, 1024], F32, tag="st", name=f"psum_{qc}_{hp}")
                nc.tensor.matmul(
                    psum[0:1, 0:512], ones_col[:], acc[:, 0:512],
                    start=True, stop=True,
                )
                nc.tensor.matmul(
                    psum[0:1, 512:1024], ones_col[:], acc[:, 512:1024],
                    start=True, stop=True,
                )
                rec = accp.tile([1, 1024], BF16, tag="rec")
                with nc.allow_low_precision(reason="softmax scale in bf16"):
                    nc.vector.reciprocal(rec[:], psum[0:1, 0:1024])
                rbc = accp.tile([128, 1024], BF16, tag="rbc")
                nc.gpsimd.partition_broadcast(rbc[:], rec[:])
                nc.vector.tensor_mul(
                    ot_sb[:, h0, bass.ts(qc, 512)], po0[:], rbc[:, 0:512]
                )
                nc.vector.tensor_mul(
                    ot_sb[:, h1, bass.ts(qc, 512)], po1[:], rbc[:, 512:1024]
                )

            for qc in range(TQC):
                if qc + 1 < TQC:
                    load_xq(qc + 1, nc.gpsimd)
                    for hh in range(HG):
                        pending_qp.extend(qproj_thunks(qc + 1, hh))
                attn_block(qc, 0, qp_gate=8 if qc + 1 < TQC else S128,
                           op_gate=4)
                attn_block(qc, 1, qp_gate=0)
                drain_all()
                pending_op.extend(oproj_thunks(qc))
            drain_all()

        if repeat == 1:
            body()
        else:
            with tc.For_i(0, repeat):
                body()

    nc.compile()
    return nc


def _shard_inputs(Xq, Xkv, q_positions, kv_positions, Wq, Wk, Wv, Wo):
    """Build per-core input maps. Core c: batch c//4, kv-group c%4."""
    D = Xq.shape[2]
    half = H // 2
    frac = 2.0 * np.arange(half, dtype=np.float32) / H
    ts = (MIN_TS * (MAX_TS / MIN_TS) ** frac).astype(np.float32)

    def tables(pos):
        s = pos.astype(np.float32)[None, :] / ts[:, None]
        return np.cos(s).astype(np.float32), np.sin(s).astype(np.float32)

    DC = D // 128

    def chunked_xT(X):
        # [L, D] -> X.T laid out as [L//512, 128, DC, 512]: contiguous per partition
        xt = np.ascontiguousarray(X.T).astype(NP_F16)  # [D, L]
        L = X.shape[0]
        return np.ascontiguousarray(
            xt.reshape(DC, 128, L // 512, 512).transpose(2, 1, 0, 3)
        )

    def chunked_w(W, m):
        # [D, m] -> [128, DC, m]
        return np.ascontiguousarray(
            W.reshape(DC, 128, m).transpose(1, 0, 2)
        ).astype(NP_F16)

    in_maps = []
    for c in range(8):
        b, g = c // 4, c % 4
        cq, sq = tables(q_positions[b])
        ck, sk = tables(kv_positions[b])
        in_maps.append(
            {
                "XqT": chunked_xT(Xq[b]),
                "XkvT": chunked_xT(Xkv[b]),
                "Wq": chunked_w(Wq[:, HG * g : HG * (g + 1), :].reshape(D, HD), HD),
                "Wk": chunked_w(Wk[:, g, :], H),
                "Wv": chunked_w(Wv[:, g, :], H),
                # Wo [HG, 128, D] -> [128, HG, D]: wo_sb[h, hh, d] = Wo[g*HG+hh, h, d]
                "Wo": np.ascontiguousarray(
                    Wo[HG * g : HG * (g + 1)].transpose(1, 0, 2)
                ).astype(NP_F16),
                "cos_q": cq, "sin_q": sq, "cos_k": ck, "sin_k": sk,
            }
        )
    return in_maps


_NC_CACHE = {}


def kernel(Xq, Xkv, q_positions, kv_positions, Wq, Wk, Wv, Wo):
    key = ("full", 1)
    if key not in _NC_CACHE:
        _NC_CACHE[key] = build()
    nc = _NC_CACHE[key]
    in_maps = _shard_inputs(Xq, Xkv, q_positions, kv_positions, Wq, Wk, Wv, Wo)
    res = run_bass_kernel_spmd(nc, in_maps, core_ids=list(range(8)))
    T, D = Xq.shape[1], Xq.shape[2]
    out = np.zeros((B, T, D), dtype=np.float32)
    for c in range(8):
        out[c // 4] += res.results[c]["out"].astype(np.float32)
    return out
